# revision 13
# baseline (speedup 1.0000x reference)
"""AttentionWithRoPE on 8 trn2 NeuronCores.

Sharding (tensor-parallel over heads x data-parallel over batch):
  core c -> batch b = c // 4, head group g = c % 4 (heads [4g, 4g+4)).
Each core computes q/k/v projections for its 4 heads (columns
[512g, 512g+512) of Wq/Wk/Wv), causal attention with RoPE, and the
partial o_proj contribution  attn_out_local @ Wo[512g:512g+512, :].
The host gather sums the 4 partials per batch (row-parallel linear).

v2 design (vs baseline): all matmul inputs in bf16 (measured pipeline
rel-err ~3.6e-3 vs 2e-2 budget), which halves DMA+SBUF so every weight
is resident and qT never spills to DRAM. Single fused j-loop
(projections -> attention -> o_proj per 512-query block) keeps the PE
stream continuous (p-state ramp). Causal diagonal blocks compute only
the valid q-suffix (bf16 runs full rate at any output width). Softmax
normalization tail (reciprocal-approx -> PE broadcast -> eviction
multiply) of head h is emitted during head h+1's attention so the PE
never waits on it; o_proj accumulates heads in order so the last
head's tail hides under the first 12 o_proj matmuls. o_proj PSUM is
DMA'd straight to DRAM (no SBUF bounce).

Per-core PE budget @2.4GHz: qkv 164us, scores+AV+rowsum ~88us (causal
trimmed), o_proj 55us => ~306us floor.
"""

import os
import sys

for _p in ("/opt/trn_rl_repo", "/root/.axon_site/_ro/trn_rl_repo"):
    if _p not in sys.path:
        sys.path.insert(0, _p)

import numpy as np
import ml_dtypes

import concourse.bass as bass
import concourse.tile as tile
from concourse import bacc, mybir
from concourse.bass_utils import run_bass_kernel_spmd

LAST_EXEC_NS = None
LAST_TRACE = None

f32 = mybir.dt.float32
f32r = mybir.dt.float32r
bf16 = mybir.dt.bfloat16
EXP = mybir.ActivationFunctionType.Exp
COPY = mybir.ActivationFunctionType.Copy

B = 2
S = 2048
E = 2048
D = 128
HL = 4          # local heads per core
EL = HL * D     # 512, local projection width
NB = S // 512   # 4 query/key 512-blocks
EC = E // 128   # 16 contraction chunks
SCALE = float(1.0 / np.sqrt(D))

_CACHE = {}


def _build():
    from contextlib import ExitStack

    nc = bacc.Bacc("TRN2", target_bir_lowering=False, debug=False, num_devices=8)

    HST = nc.dram_tensor("hsT", [E, S], bf16, kind="ExternalInput")
    WQ = nc.dram_tensor("wq", [E, EL], bf16, kind="ExternalInput")
    WK = nc.dram_tensor("wk", [E, EL], bf16, kind="ExternalInput")
    WV = nc.dram_tensor("wv", [E, EL], bf16, kind="ExternalInput")
    WO = nc.dram_tensor("wo", [EL, E], bf16, kind="ExternalInput")
    COS = nc.dram_tensor("cosT", [D, S], bf16, kind="ExternalInput")
    SIN = nc.dram_tensor("sinTs", [D, S], bf16, kind="ExternalInput")  # sign-folded
    TRI = nc.dram_tensor("tri", [128, 128], bf16, kind="ExternalInput")
    ONEC = nc.dram_tensor("onec", [128, 1], bf16, kind="ExternalInput")
    ONER = nc.dram_tensor("oner", [1, 128], bf16, kind="ExternalInput")
    OUT = nc.dram_tensor("out", [S, E], f32, kind="ExternalOutput")

    with tile.TileContext(nc) as tc, nc.allow_low_precision("bf16 compute by design"):
        with ExitStack() as octx:
            res = octx.enter_context(tc.tile_pool(name="res", bufs=1))
            kT = [res.tile([128, S], bf16, tag=f"kT{h}", name=f"kT{h}") for h in range(HL)]
            qT = [res.tile([128, S], bf16, tag=f"qT{h}", name=f"qT{h}") for h in range(HL)]
            v_sb = res.tile([128, NB * 4, EL], bf16, tag="v")
            wk_sb = res.tile([128, EC, EL], bf16, tag="wk")
            wq_sb = res.tile([128, EC, EL], bf16, tag="wq")
            wv_sb = res.tile([128, EC, EL], bf16, tag="wv")
            wo_sb = res.tile([128, HL, E], bf16, tag="wo")
            cos_sb = res.tile([128, S], bf16, tag="cos")
            sin_sb = res.tile([128, S], bf16, tag="sin")
            tri = res.tile([128, 128], bf16, tag="tri")
            onec = res.tile([128, 1], bf16, tag="onec")
            oner = res.tile([1, 128], bf16, tag="oner")

            def ld_w(dst, srcT, half):
                src = srcT[half * 1024:(half + 1) * 1024, :]
                nc.sync.dma_start(
                    dst[:, half * 8:(half + 1) * 8, :],
                    src.rearrange("(c p) m -> p c m", p=128),
                )

            hsp = octx.enter_context(tc.tile_pool(name="hs", bufs=2))
            rp = octx.enter_context(tc.tile_pool(name="rope", bufs=2))
            exp_p = octx.enter_context(tc.tile_pool(name="exp", bufs=3))
            onp = octx.enter_context(tc.tile_pool(name="on", bufs=2))
            pp = octx.enter_context(tc.tile_pool(name="pp", bufs=1, space="PSUM"))

            def rope_evict(dst, ps, cos_t, sin_t):
                # dst = raw*cosT + rot(raw)*sinT_signed  (bf16)
                raw = rp.tile([128, 512], bf16, tag="raw", name="raw")
                nc.scalar.activation(raw[:], ps[:], COPY)
                rot = rp.tile([128, 512], bf16, tag="rot", name="rot")
                nc.sync.dma_start(rot[0:64, :], raw[64:128, :])
                nc.sync.dma_start(rot[64:128, :], raw[0:64, :])
                t1 = rp.tile([128, 512], bf16, tag="t1", name="t1")
                nc.vector.tensor_mul(t1[:], raw[:], cos_t)
                nc.vector.tensor_mul(dst, rot[:], sin_t)
                nc.vector.tensor_add(dst, dst, t1[:])

            hs_tiles = {}

            def emit_hs_load(j):
                hs = hsp.tile([128, EC, 512], bf16, tag="hs", name="hs")
                for half in range(2):
                    src = HST[half * 1024:(half + 1) * 1024, j * 512:(j + 1) * 512]
                    nc.sync.dma_start(
                        hs[:, half * 8:(half + 1) * 8, :],
                        src.rearrange("(c p) s -> p c s", p=128),
                    )
                hs_tiles[j] = hs

            # Input loads, ordered so the first consumers (k projections
            # of j=0, then q, then v) wait the least.
            emit_hs_load(0)
            ld_w(wk_sb, WK, 0)
            ld_w(wk_sb, WK, 1)
            ld_w(wq_sb, WQ, 0)
            ld_w(wq_sb, WQ, 1)
            nc.sync.dma_start(cos_sb[:], COS[:])
            nc.sync.dma_start(sin_sb[:], SIN[:])
            ld_w(wv_sb, WV, 0)
            ld_w(wv_sb, WV, 1)
            nc.sync.dma_start(onec[:], ONEC[:])
            nc.sync.dma_start(oner[:], ONER[:])
            nc.sync.dma_start(tri[:], TRI[:])
            for hh in range(HL):
                nc.sync.dma_start(
                    wo_sb[:, hh, :],
                    WO[hh * 128:(hh + 1) * 128, :],
                )

            def emit_proj(j):
                hs = hs_tiles[j]
                cos_t = cos_sb[:, j * 512:(j + 1) * 512]
                sin_t = sin_sb[:, j * 512:(j + 1) * 512]
                # K then Q projections (+fused RoPE eviction)
                for wsb, dstT in ((wk_sb, kT), (wq_sb, qT)):
                    for h in range(HL):
                        ps = pp.tile([128, 512], f32, tag="kq", bufs=3, name="kqps")
                        for e in range(EC):
                            nc.tensor.matmul(
                                ps[:],
                                wsb[:, e, h * 128:(h + 1) * 128],
                                hs[:, e, :],
                                start=(e == 0),
                                stop=(e == EC - 1),
                            )
                        rope_evict(
                            dstT[h][:, j * 512:(j + 1) * 512], ps, cos_t, sin_t
                        )
                # V
                for i in range(4):
                    vp = pp.tile([128, EL], f32, tag="vsc", bufs=3, name="vps")
                    for e in range(EC):
                        nc.tensor.matmul(
                            vp[:],
                            hs[:, e, i * 128:(i + 1) * 128],
                            wv_sb[:, e, :],
                            start=(e == 0),
                            stop=(e == EC - 1),
                        )
                    nc.scalar.activation(v_sb[:, j * 4 + i, :], vp[:], COPY)

            def emit_attn(j, o_norm):
                nkb = 4 * j + 4
                tails = [None] * HL

                def emit_tail(h):
                    av, lt = tails[h]
                    recip = onp.tile([128, 512], f32, tag="recip", name="recip")
                    nc.vector.reciprocal_approx_fast(
                        out=recip[0:1, :], in_=lt[0:1, :]
                    )
                    recip_b = onp.tile([128, 512], bf16, tag="recipb", name="recipb")
                    nc.scalar.activation(recip_b[0:1, :], recip[0:1, :], COPY)
                    bc_ps = pp.tile([128, 512], f32, tag="vsc", bufs=3, name="bcps")
                    nc.tensor.matmul(
                        bc_ps[:], oner[:], recip_b[0:1, :], start=True, stop=True
                    )
                    bc_sb = onp.tile([128, 512], bf16, tag="bcsb", name="bcsb")
                    nc.scalar.activation(bc_sb[:], bc_ps[:], COPY)
                    on = onp.tile([128, 512], bf16, tag="onorm", bufs=5, name="onorm")
                    nc.vector.tensor_mul(on[:], av[:], bc_sb[:])
                    o_norm[h] = on

                for h in range(HL):
                    av = pp.tile([128, 512], f32, tag="av", bufs=2, name="avps")
                    lt = pp.tile([128, 512], f32, tag="kq", bufs=3, name="lps")
                    exs = [None] * nkb
                    # Off-diagonal exp tiles are rowsummed in pairs (one
                    # DVE pre-add halves the PE rowsum matmuls); the first
                    # l matmul is the kb=1 pair (j>0) or the kb=0 diagonal
                    # (j=0).
                    l_started = [False]

                    def emit_sc(kb):
                        m = kb - 4 * j
                        off = 128 * m if m >= 0 else 0
                        w = 512 - off
                        sc = pp.tile([128, 512], f32, tag="vsc", bufs=3, name="scps")
                        nc.tensor.matmul(
                            sc[:, 0:w],
                            kT[h][:, kb * 128:(kb + 1) * 128],
                            qT[h][:, j * 512 + off:(j + 1) * 512],
                            start=True,
                            stop=True,
                        )
                        ex = exp_p.tile([128, 512], bf16, tag="ex", bufs=4, name="ex")
                        nc.scalar.activation(ex[:, 0:w], sc[:, 0:w], EXP, scale=SCALE)
                        if m >= 0:
                            nc.vector.tensor_mul(ex[:, 0:128], ex[:, 0:128], tri[:])
                        exs[kb] = ex

                    def emit_avl(kb):
                        m = kb - 4 * j
                        off = 128 * m if m >= 0 else 0
                        w = 512 - off
                        nc.tensor.matmul(
                            av[:, off:512],
                            v_sb[:, kb, h * 128:(h + 1) * 128],
                            exs[kb][:, 0:w],
                            start=(kb == 0),
                            stop=(kb == nkb - 1),
                            skip_group_check=True,
                        )
                        if m < 0 and kb % 2 == 0:
                            return  # rowsummed with its pair at kb+1
                        if m < 0:
                            pair = exp_p.tile(
                                [128, 512], bf16, tag="expair", bufs=2, name="expair"
                            )
                            nc.vector.tensor_add(
                                pair[:], exs[kb - 1][:], exs[kb][:]
                            )
                            l_in = pair[:, 0:512]
                        else:
                            l_in = exs[kb][:, 0:w]
                        nc.tensor.matmul(
                            lt[0:1, off:512],
                            onec[:],
                            l_in,
                            start=(not l_started[0]),
                            stop=(kb == nkb - 1),
                            skip_group_check=True,
                        )
                        l_started[0] = True

                    emit_sc(0)
                    for kb in range(1, nkb):
                        emit_sc(kb)
                        emit_avl(kb - 1)
                    emit_avl(nkb - 1)
                    tails[h] = (av, lt)
                    if h > 0:
                        emit_tail(h - 1)
                emit_tail(HL - 1)

            def emit_oproj(j, o_norm):
                for i in range(4):
                    orow = onp.tile([128, E], f32, tag="orow", bufs=2, name="orow")
                    for n in range(4):
                        op = pp.tile([128, 512], f32, tag="kq", bufs=3, name="opps")
                        for h in range(HL):
                            nc.tensor.matmul(
                                op[:],
                                o_norm[h][:, i * 128:(i + 1) * 128],
                                wo_sb[:, h, n * 512:(n + 1) * 512],
                                start=(h == 0),
                                stop=(h == HL - 1),
                            )
                        nc.vector.tensor_copy(
                            orow[:, n * 512:(n + 1) * 512], op[:]
                        )
                        nc.sync.dma_start(
                            OUT[
                                j * 512 + i * 128:j * 512 + (i + 1) * 128,
                                n * 512:(n + 1) * 512,
                            ],
                            orow[:, n * 512:(n + 1) * 512],
                        )

            # proj(j+1) sits between attn(j) and o_proj(j): the last
            # head's normalization tail hides under 41us of projection
            # matmuls, and o_proj(j) never stalls the PE.
            emit_hs_load(0)
            emit_proj(0)
            for j in range(NB):
                o_norm = [None] * HL
                if j + 1 < NB:
                    emit_hs_load(j + 1)
                emit_attn(j, o_norm)
                if j + 1 < NB:
                    emit_proj(j + 1)
                emit_oproj(j, o_norm)

    nc.compile()
    return nc


def _get_nc():
    if "nc" not in _CACHE:
        _CACHE["nc"] = _build()
    return _CACHE["nc"]


def kernel(hidden_states, cos, sin, Wq, Wk, Wv, Wo):
    bf = ml_dtypes.bfloat16
    hidden_states = np.asarray(hidden_states, dtype=np.float32)
    cos = np.asarray(cos, dtype=np.float32)
    sin = np.asarray(sin, dtype=np.float32)
    Wq = np.asarray(Wq, dtype=np.float32)
    Wk = np.asarray(Wk, dtype=np.float32)
    Wv = np.asarray(Wv, dtype=np.float32)
    Wo = np.asarray(Wo, dtype=np.float32)

    nc = _get_nc()

    sk = np.arange(128)[:, None]
    sq = np.arange(128)[None, :]
    tri = (sq >= sk).astype(bf)
    onec = np.ones((128, 1), dtype=bf)
    oner = np.ones((1, 128), dtype=bf)

    hsT = [np.ascontiguousarray(hidden_states[b].T).astype(bf) for b in range(B)]
    cosT = [np.ascontiguousarray(cos[b].T).astype(bf) for b in range(B)]
    sinTs = []
    for b in range(B):
        s = np.ascontiguousarray(sin[b].T)
        s[:64] *= -1.0
        sinTs.append(s.astype(bf))

    in_maps = []
    for c in range(8):
        b, g = c // 4, c % 4
        cols = slice(512 * g, 512 * (g + 1))
        in_maps.append({
            "hsT": hsT[b],
            "wq": np.ascontiguousarray(Wq[:, cols]).astype(bf),
            "wk": np.ascontiguousarray(Wk[:, cols]).astype(bf),
            "wv": np.ascontiguousarray(Wv[:, cols]).astype(bf),
            "wo": np.ascontiguousarray(Wo[cols, :]).astype(bf),
            "cosT": cosT[b],
            "sinTs": sinTs[b],
            "tri": tri,
            "onec": onec,
            "oner": oner,
        })

    global LAST_EXEC_NS, LAST_TRACE
    trace = bool(int(os.environ.get("KTRACE", "0")))
    tc_env = os.environ.get("KTRACE_CORES", "0")
    trace_cores = [int(x) for x in tc_env.split(",")] if trace else None
    res = run_bass_kernel_spmd(
        nc, in_maps, core_ids=list(range(8)),
        trace=trace, trace_cores=trace_cores,
    )
    if res.exec_time_ns is not None:
        LAST_EXEC_NS = res.exec_time_ns
        LAST_TRACE = res.instructions_and_trace
        print(f"[kernel] exec_time_ns={res.exec_time_ns} "
              f"mean={res.mean_exec_time_ns} max_core={res.max_exec_time_core_id}")
        if res.instructions_and_trace:
            print(f"[kernel] trace: {res.instructions_and_trace[1]}")

    out = np.empty((B, S, E), dtype=np.float32)
    for b in range(B):
        acc = res.results[4 * b]["out"].astype(np.float32)
        for g in range(1, 4):
            acc = acc + res.results[4 * b + g]["out"]
        out[b] = acc
    return out


# revision 17
# speedup vs baseline: 1.2378x; 1.2378x over previous
"""AttentionWithRoPE on 8 trn2 NeuronCores.

Sharding (tensor-parallel over heads x data-parallel over batch):
  core c -> batch b = c // 4, head group g = c % 4 (heads [4g, 4g+4)).
Each core computes q/k/v projections for its 4 heads (columns
[512g, 512g+512) of Wq/Wk/Wv), causal attention with RoPE, and the
partial o_proj contribution  attn_out_local @ Wo[512g:512g+512, :].
The host gather sums the 4 partials per batch (row-parallel linear).

v2 design (vs baseline): all matmul inputs in bf16 (measured pipeline
rel-err ~3.6e-3 vs 2e-2 budget), which halves DMA+SBUF so every weight
is resident and qT never spills to DRAM. Single fused j-loop
(projections -> attention -> o_proj per 512-query block) keeps the PE
stream continuous (p-state ramp). Causal diagonal blocks compute only
the valid q-suffix (bf16 runs full rate at any output width). Softmax
normalization tail (reciprocal-approx -> PE broadcast -> eviction
multiply) of head h is emitted during head h+1's attention so the PE
never waits on it; o_proj accumulates heads in order so the last
head's tail hides under the first 12 o_proj matmuls. o_proj PSUM is
DMA'd straight to DRAM (no SBUF bounce).

Per-core PE budget @2.4GHz: qkv 164us, scores+AV+rowsum ~88us (causal
trimmed), o_proj 55us => ~306us floor.
"""

import os
import sys

for _p in ("/opt/trn_rl_repo", "/root/.axon_site/_ro/trn_rl_repo"):
    if _p not in sys.path:
        sys.path.insert(0, _p)

import numpy as np
import ml_dtypes

import concourse.bass as bass
import concourse.tile as tile
from concourse import bacc, mybir
from concourse.bass_utils import run_bass_kernel_spmd

LAST_EXEC_NS = None
LAST_TRACE = None

f32 = mybir.dt.float32
f32r = mybir.dt.float32r
bf16 = mybir.dt.bfloat16
EXP = mybir.ActivationFunctionType.Exp
COPY = mybir.ActivationFunctionType.Copy

B = 2
S = 2048
E = 2048
D = 128
HL = 4          # local heads per core
EL = HL * D     # 512, local projection width
NB = S // 512   # 4 query/key 512-blocks
EC = E // 128   # 16 contraction chunks
SCALE = float(1.0 / np.sqrt(D))

_CACHE = {}


def _build():
    from contextlib import ExitStack

    nc = bacc.Bacc("TRN2", target_bir_lowering=False, debug=False, num_devices=8)

    HST = nc.dram_tensor("hsT", [E, S], bf16, kind="ExternalInput")
    WQ = nc.dram_tensor("wq", [E, EL], bf16, kind="ExternalInput")
    WK = nc.dram_tensor("wk", [E, EL], bf16, kind="ExternalInput")
    WV = nc.dram_tensor("wv", [E, EL], bf16, kind="ExternalInput")
    WO = nc.dram_tensor("wo", [EL, E], bf16, kind="ExternalInput")
    COS = nc.dram_tensor("cosT", [D, S], bf16, kind="ExternalInput")
    SIN = nc.dram_tensor("sinTs", [D, S], bf16, kind="ExternalInput")  # sign-folded
    TRI = nc.dram_tensor("tri", [128, 128], bf16, kind="ExternalInput")
    ONEC = nc.dram_tensor("onec", [128, 1], bf16, kind="ExternalInput")
    ONER = nc.dram_tensor("oner", [1, 128], bf16, kind="ExternalInput")
    OUT = nc.dram_tensor("out", [S, E], f32, kind="ExternalOutput")

    with tile.TileContext(nc) as tc, nc.allow_low_precision("bf16 compute by design"):
        with ExitStack() as octx:
            res = octx.enter_context(tc.tile_pool(name="res", bufs=1))
            kT = [res.tile([128, S], bf16, tag=f"kT{h}", name=f"kT{h}") for h in range(HL)]
            qT = [res.tile([128, S], bf16, tag=f"qT{h}", name=f"qT{h}") for h in range(HL)]
            v_sb = res.tile([128, NB * 4, EL], bf16, tag="v")
            wk_sb = res.tile([128, EC, EL], bf16, tag="wk")
            wq_sb = res.tile([128, EC, EL], bf16, tag="wq")
            wv_sb = res.tile([128, EC, EL], bf16, tag="wv")
            wo_sb = res.tile([128, HL, E], bf16, tag="wo")
            cos_sb = res.tile([128, S], bf16, tag="cos")
            sin_sb = res.tile([128, S], bf16, tag="sin")
            tri = res.tile([128, 128], bf16, tag="tri")
            onec = res.tile([128, 1], bf16, tag="onec")
            oner = res.tile([1, 128], bf16, tag="oner")

            def ld_w(dst, srcT, half):
                src = srcT[half * 1024:(half + 1) * 1024, :]
                nc.sync.dma_start(
                    dst[:, half * 8:(half + 1) * 8, :],
                    src.rearrange("(c p) m -> p c m", p=128),
                )

            hsp = octx.enter_context(tc.tile_pool(name="hs", bufs=2))
            rp = octx.enter_context(tc.tile_pool(name="rope", bufs=2))
            exp_p = octx.enter_context(tc.tile_pool(name="exp", bufs=3))
            onp = octx.enter_context(tc.tile_pool(name="on", bufs=2))
            pp = octx.enter_context(tc.tile_pool(name="pp", bufs=1, space="PSUM"))

            def rope_evict(dst, ps, cos_t, sin_t):
                # dst = raw*cosT + rot(raw)*sinT_signed  (bf16)
                # rot DMAs go on the ACT hardware queue so they never sit
                # behind bulk weight/hs prefetches on the SP queue.
                raw = rp.tile([128, 512], bf16, tag="raw", name="raw")
                nc.scalar.activation(raw[:], ps[:], COPY)
                rot = rp.tile([128, 512], bf16, tag="rot", name="rot")
                nc.scalar.dma_start(rot[0:64, :], raw[64:128, :])
                nc.scalar.dma_start(rot[64:128, :], raw[0:64, :])
                t1 = rp.tile([128, 512], bf16, tag="t1", name="t1")
                nc.vector.tensor_mul(t1[:], raw[:], cos_t)
                nc.vector.tensor_mul(dst, rot[:], sin_t)
                nc.vector.tensor_add(dst, dst, t1[:])

            hs_tiles = {}

            def emit_hs_load(j):
                hs = hsp.tile([128, EC, 512], bf16, tag="hs", name="hs")
                for half in range(2):
                    src = HST[half * 1024:(half + 1) * 1024, j * 512:(j + 1) * 512]
                    nc.sync.dma_start(
                        hs[:, half * 8:(half + 1) * 8, :],
                        src.rearrange("(c p) s -> p c s", p=128),
                    )
                hs_tiles[j] = hs

            # Input loads on the SP FIFO queue, ordered so the first
            # consumers (k projections of j=0, then q, then v) wait the
            # least: interleave hs(0)/wk halves first.
            hs0 = hsp.tile([128, EC, 512], bf16, tag="hs", name="hs")
            hs_tiles[0] = hs0
            for half in range(2):
                nc.sync.dma_start(
                    hs0[:, half * 8:(half + 1) * 8, :],
                    HST[half * 1024:(half + 1) * 1024, 0:512].rearrange(
                        "(c p) s -> p c s", p=128
                    ),
                )
                ld_w(wk_sb, WK, half)
            ld_w(wq_sb, WQ, 0)
            ld_w(wq_sb, WQ, 1)
            nc.sync.dma_start(cos_sb[:], COS[:])
            nc.sync.dma_start(sin_sb[:], SIN[:])
            ld_w(wv_sb, WV, 0)
            ld_w(wv_sb, WV, 1)
            nc.sync.dma_start(onec[:], ONEC[:])
            nc.sync.dma_start(oner[:], ONER[:])
            nc.sync.dma_start(tri[:], TRI[:])
            for hh in range(HL):
                nc.sync.dma_start(
                    wo_sb[:, hh, :],
                    WO[hh * 128:(hh + 1) * 128, :],
                )

            def emit_proj(j):
                hs = hs_tiles[j]
                cos_t = cos_sb[:, j * 512:(j + 1) * 512]
                sin_t = sin_sb[:, j * 512:(j + 1) * 512]
                # K then Q projections (+fused RoPE eviction)
                for wsb, dstT in ((wk_sb, kT), (wq_sb, qT)):
                    for h in range(HL):
                        ps = pp.tile([128, 512], f32, tag="kq", bufs=3, name="kqps")
                        for e in range(EC):
                            nc.tensor.matmul(
                                ps[:],
                                wsb[:, e, h * 128:(h + 1) * 128],
                                hs[:, e, :],
                                start=(e == 0),
                                stop=(e == EC - 1),
                            )
                        rope_evict(
                            dstT[h][:, j * 512:(j + 1) * 512], ps, cos_t, sin_t
                        )
                # V
                for i in range(4):
                    vp = pp.tile([128, EL], f32, tag="vsc", bufs=3, name="vps")
                    for e in range(EC):
                        nc.tensor.matmul(
                            vp[:],
                            hs[:, e, i * 128:(i + 1) * 128],
                            wv_sb[:, e, :],
                            start=(e == 0),
                            stop=(e == EC - 1),
                        )
                    nc.scalar.activation(v_sb[:, j * 4 + i, :], vp[:], COPY)

            def emit_attn(j, o_norm):
                nkb = 4 * j + 4
                tails = [None] * HL

                def emit_tail(h):
                    av, lt = tails[h]
                    recip = onp.tile([128, 512], f32, tag="recip", name="recip")
                    nc.vector.reciprocal_approx_fast(
                        out=recip[0:1, :], in_=lt[0:1, :]
                    )
                    recip_b = onp.tile([128, 512], bf16, tag="recipb", name="recipb")
                    nc.scalar.activation(recip_b[0:1, :], recip[0:1, :], COPY)
                    bc_ps = pp.tile([128, 512], f32, tag="vsc", bufs=3, name="bcps")
                    nc.tensor.matmul(
                        bc_ps[:], oner[:], recip_b[0:1, :], start=True, stop=True
                    )
                    bc_sb = onp.tile([128, 512], bf16, tag="bcsb", name="bcsb")
                    nc.scalar.activation(bc_sb[:], bc_ps[:], COPY)
                    on = onp.tile([128, 512], bf16, tag="onorm", bufs=5, name="onorm")
                    nc.vector.tensor_mul(on[:], av[:], bc_sb[:])
                    o_norm[h] = on

                for h in range(HL):
                    av = pp.tile([128, 512], f32, tag="av", bufs=2, name="avps")
                    lt = pp.tile([128, 512], f32, tag="kq", bufs=3, name="lps")
                    exs = [None] * nkb
                    # Off-diagonal exp tiles are pre-added in pairs on the
                    # DVE; their rowsum matmuls are deferred to the end of
                    # the head so the PE never waits on the adds. Diagonal
                    # rowsums run inline (same dep as the AV matmul). The
                    # PSUM series starts at the m=0 diagonal (full width)
                    # and stops on the last deferred pair.
                    pairs = []

                    def emit_sc(kb):
                        m = kb - 4 * j
                        off = 128 * m if m >= 0 else 0
                        w = 512 - off
                        sc = pp.tile([128, 512], f32, tag="vsc", bufs=3, name="scps")
                        nc.tensor.matmul(
                            sc[:, 0:w],
                            kT[h][:, kb * 128:(kb + 1) * 128],
                            qT[h][:, j * 512 + off:(j + 1) * 512],
                            start=True,
                            stop=True,
                        )
                        ex = exp_p.tile([128, 512], bf16, tag="ex", bufs=4, name="ex")
                        nc.scalar.activation(ex[:, 0:w], sc[:, 0:w], EXP, scale=SCALE)
                        if m >= 0:
                            nc.vector.tensor_mul(ex[:, 0:128], ex[:, 0:128], tri[:])
                        exs[kb] = ex

                    def emit_avl(kb):
                        m = kb - 4 * j
                        off = 128 * m if m >= 0 else 0
                        w = 512 - off
                        nc.tensor.matmul(
                            av[:, off:512],
                            v_sb[:, kb, h * 128:(h + 1) * 128],
                            exs[kb][:, 0:w],
                            start=(kb == 0),
                            stop=(kb == nkb - 1),
                            skip_group_check=True,
                        )
                        if m < 0:
                            if kb % 2 == 1:
                                pair = exp_p.tile(
                                    [128, 512], bf16, tag="expair", bufs=6,
                                    name="expair",
                                )
                                nc.vector.tensor_add(
                                    pair[:], exs[kb - 1][:], exs[kb][:]
                                )
                                pairs.append(pair)
                        else:
                            nc.tensor.matmul(
                                lt[0:1, off:512],
                                onec[:],
                                exs[kb][:, 0:w],
                                start=(m == 0),
                                stop=(m == 3 and j == 0),
                                skip_group_check=True,
                            )

                    emit_sc(0)
                    for kb in range(1, nkb):
                        emit_sc(kb)
                        emit_avl(kb - 1)
                    emit_avl(nkb - 1)
                    for pi, pair in enumerate(pairs):
                        nc.tensor.matmul(
                            lt[0:1, :],
                            onec[:],
                            pair[:],
                            start=False,
                            stop=(pi == len(pairs) - 1),
                            skip_group_check=True,
                        )
                    tails[h] = (av, lt)
                    if h > 0:
                        emit_tail(h - 1)
                emit_tail(HL - 1)

            def emit_oproj(j, o_norm):
                for i in range(4):
                    orow = onp.tile([128, E], f32, tag="orow", bufs=2, name="orow")
                    for n in range(4):
                        op = pp.tile([128, 512], f32, tag="kq", bufs=3, name="opps")
                        for h in range(HL):
                            nc.tensor.matmul(
                                op[:],
                                o_norm[h][:, i * 128:(i + 1) * 128],
                                wo_sb[:, h, n * 512:(n + 1) * 512],
                                start=(h == 0),
                                stop=(h == HL - 1),
                            )
                        nc.vector.tensor_copy(
                            orow[:, n * 512:(n + 1) * 512], op[:]
                        )
                        nc.sync.dma_start(
                            OUT[
                                j * 512 + i * 128:j * 512 + (i + 1) * 128,
                                n * 512:(n + 1) * 512,
                            ],
                            orow[:, n * 512:(n + 1) * 512],
                        )

            # proj(j+1) sits between attn(j) and o_proj(j): the last
            # head's normalization tail hides under 41us of projection
            # matmuls, and o_proj(j) never stalls the PE.
            emit_proj(0)
            for j in range(NB):
                o_norm = [None] * HL
                if j + 1 < NB:
                    emit_hs_load(j + 1)
                emit_attn(j, o_norm)
                if j + 1 < NB:
                    emit_proj(j + 1)
                emit_oproj(j, o_norm)

    nc.compile()
    return nc


def _get_nc():
    if "nc" not in _CACHE:
        _CACHE["nc"] = _build()
    return _CACHE["nc"]


def kernel(hidden_states, cos, sin, Wq, Wk, Wv, Wo):
    bf = ml_dtypes.bfloat16
    hidden_states = np.asarray(hidden_states, dtype=np.float32)
    cos = np.asarray(cos, dtype=np.float32)
    sin = np.asarray(sin, dtype=np.float32)
    Wq = np.asarray(Wq, dtype=np.float32)
    Wk = np.asarray(Wk, dtype=np.float32)
    Wv = np.asarray(Wv, dtype=np.float32)
    Wo = np.asarray(Wo, dtype=np.float32)

    nc = _get_nc()

    sk = np.arange(128)[:, None]
    sq = np.arange(128)[None, :]
    tri = (sq >= sk).astype(bf)
    onec = np.ones((128, 1), dtype=bf)
    oner = np.ones((1, 128), dtype=bf)

    hsT = [np.ascontiguousarray(hidden_states[b].T).astype(bf) for b in range(B)]
    cosT = [np.ascontiguousarray(cos[b].T).astype(bf) for b in range(B)]
    sinTs = []
    for b in range(B):
        s = np.ascontiguousarray(sin[b].T)
        s[:64] *= -1.0
        sinTs.append(s.astype(bf))

    in_maps = []
    for c in range(8):
        b, g = c // 4, c % 4
        cols = slice(512 * g, 512 * (g + 1))
        in_maps.append({
            "hsT": hsT[b],
            "wq": np.ascontiguousarray(Wq[:, cols]).astype(bf),
            "wk": np.ascontiguousarray(Wk[:, cols]).astype(bf),
            "wv": np.ascontiguousarray(Wv[:, cols]).astype(bf),
            "wo": np.ascontiguousarray(Wo[cols, :]).astype(bf),
            "cosT": cosT[b],
            "sinTs": sinTs[b],
            "tri": tri,
            "onec": onec,
            "oner": oner,
        })

    global LAST_EXEC_NS, LAST_TRACE
    trace = bool(int(os.environ.get("KTRACE", "0")))
    tc_env = os.environ.get("KTRACE_CORES", "0")
    trace_cores = [int(x) for x in tc_env.split(",")] if trace else None
    res = run_bass_kernel_spmd(
        nc, in_maps, core_ids=list(range(8)),
        trace=trace, trace_cores=trace_cores,
    )
    if res.exec_time_ns is not None:
        LAST_EXEC_NS = res.exec_time_ns
        LAST_TRACE = res.instructions_and_trace
        print(f"[kernel] exec_time_ns={res.exec_time_ns} "
              f"mean={res.mean_exec_time_ns} max_core={res.max_exec_time_core_id}")
        if res.instructions_and_trace:
            print(f"[kernel] trace: {res.instructions_and_trace[1]}")

    out = np.empty((B, S, E), dtype=np.float32)
    for b in range(B):
        acc = res.results[4 * b]["out"].astype(np.float32)
        for g in range(1, 4):
            acc = acc + res.results[4 * b + g]["out"]
        out[b] = acc
    return out
